# revision 13
# baseline (speedup 1.0000x reference)
"""Trainium2 Bass kernel for CrossAttentionConditionInjection.

Math: the attention keys/values come from a single condition token broadcast
across the sequence, so the scores are constant along the key axis; softmax is
exactly uniform and the attention output collapses to

    out[b, s, :] = (condition[b] @ Wv.T + bv) @ Wo.T + bo      (for every s)

independent of hidden_states / Wq / Wk / q entirely.

Sharding (2D): core i -> (batch b = i//4, output-column quarter q = i%4).
Every core computes the full v1 = cond[b] @ Wv.T + bv (Wv.T is irreducible
per-core without cross-core exchange) but only its 256-column slice of
row = v1 @ Wo.T + bo, and broadcast-writes it across all 2048 sequence
positions of its batch.  The host reassembles the column quarters.

Engine split (fp32 streams through the PE at ~4 cycles/column, so the
mat-vec bulk stays off the PE):
  stage 1 muls: ACT activation(Copy, scale=cond-per-partition), one per wv
      k-chunk, chasing the chunked wv DMA.
  stage 1 sum:  DVE rolling adds (in-place accumulate), also chasing.
  v1T:          8 tiny PE matmuls  lhsT=partial-chunk, rhs=ones column ->
                v1 landed on partitions; one DVE add folds in bv.
  v1 broadcast: single DVE copy with a step-0 AP.
  stage 2:      8 PE matmuls N=256 over the per-core Wo.T quarter + a K=1
                ones-row matmul for bo.
  output:       one DMA broadcast-writes the [128, 256] row tile 16x into
                the contiguous per-core [2048, 256] output.
"""

import numpy as np
from contextlib import ExitStack

import concourse.bass as bass
import concourse.bacc as bacc
import concourse.mybir as mybir
import concourse.tile as tile
from concourse.bass_utils import run_bass_kernel_spmd

B, S, D = 2, 2048, 1024
NCORES = 8
QCORES = NCORES // B  # cores per batch -> column quarters
QW = D // QCORES  # 256 columns per core
KC = D // 128  # 8 contraction chunks
WV_CHUNKS = 8

_cache = {}


def _build():
    f32 = mybir.dt.float32
    nc = bacc.Bacc()

    smalls = nc.dram_tensor("smalls", [128, 2 * KC], f32, kind="ExternalInput")
    wvp = nc.dram_tensor("wvp", [128, KC * D], f32, kind="ExternalInput")
    woq = nc.dram_tensor("woq", [128, KC * QW], f32, kind="ExternalInput")
    boq = nc.dram_tensor("boq", [1, QW], f32, kind="ExternalInput")
    y = nc.dram_tensor("y", [S, QW], f32, kind="ExternalOutput")

    with tile.TileContext(nc) as tc, ExitStack() as ctx:
        wv_pool = ctx.enter_context(tc.tile_pool(name="wv", bufs=1))
        wo_pool = ctx.enter_context(tc.tile_pool(name="wo", bufs=1))
        small = ctx.enter_context(tc.tile_pool(name="small", bufs=1))
        tmpp = ctx.enter_context(tc.tile_pool(name="tmpp", bufs=1))
        outp = ctx.enter_context(tc.tile_pool(name="outp", bufs=1))
        psumv = ctx.enter_context(
            tc.tile_pool(name="psumv", bufs=1, space=bass.MemorySpace.PSUM)
        )
        psum2 = ctx.enter_context(
            tc.tile_pool(name="psum2", bufs=1, space=bass.MemorySpace.PSUM)
        )
        psumj = ctx.enter_context(
            tc.tile_pool(name="psumj", bufs=1, space=bass.MemorySpace.PSUM)
        )

        ones1x128 = small.tile([1, 128], f32)
        nc.vector.memset(ones1x128[:], 1.0)
        ones_col = small.tile([128, 1], f32)
        nc.vector.memset(ones_col[:], 1.0)

        # ---- loads: smalls first, then wv chunked, then the wo quarter ----
        smalls_sb = small.tile([128, 2 * KC], f32)
        nc.sync.dma_start(smalls_sb[:], smalls[:])
        boq_sb = small.tile([1, QW], f32)
        nc.sync.dma_start(boq_sb[:], boq[:])
        condT = smalls_sb[:, 0:KC]
        bvT = smalls_sb[:, KC : 2 * KC]

        wv_sb = wv_pool.tile([128, KC * D], f32)
        wv_c = (KC * D) // WV_CHUNKS
        for c in range(WV_CHUNKS):
            eng = nc.sync if c % 2 == 0 else nc.scalar
            eng.dma_start(
                wv_sb[:, c * wv_c : (c + 1) * wv_c], wvp[:, c * wv_c : (c + 1) * wv_c]
            )
        woq_sb = wo_pool.tile([128, KC * QW], f32)
        nc.scalar.dma_start(woq_sb[:], woq[:])

        # HAM warm-keeper: sustained junk matmuls chasing each wv chunk keep
        # the PE activity window busy through the DMA phase, so the real
        # matmuls below run at 2.4 GHz instead of the cold 1.2 GHz default.
        junk_ps = psumj.tile([128, 8], f32)
        for c in range(WV_CHUNKS):
            for r in range(3):
                nc.tensor.matmul(
                    junk_ps[:],
                    wv_sb[:, c * wv_c + r * 128 : c * wv_c + r * 128 + 128],
                    condT[:, 0:8],
                    start=True,
                    stop=True,
                )

        # ---- stage 1: partial[p, c] = sum_k WvT[k*128+p, c] * cond[k*128+p]
        # ACT does the per-partition-scalar muls, DVE rolls the sum.
        tmp = tmpp.tile([128, KC * D], f32)
        for k in range(KC):
            nc.scalar.activation(
                tmp[:, k * D : (k + 1) * D],
                wv_sb[:, k * D : (k + 1) * D],
                mybir.ActivationFunctionType.Copy,
                scale=condT[:, k : k + 1],
            )
        partial = tmp[:, :D]
        for k in range(1, KC):
            nc.vector.tensor_add(partial, partial, tmp[:, k * D : (k + 1) * D])

        # ---- v1T on partitions: out[m, 0] = sum_p partial[p, j*128+m] ----
        v1T_ps = psumv.tile([128, KC], f32)
        for j in range(KC):
            nc.tensor.matmul(
                v1T_ps[:, j : j + 1],
                partial[:, j * 128 : (j + 1) * 128],
                ones_col[:],
                start=True,
                stop=True,
            )
        v1T_sb = small.tile([128, KC], f32)
        nc.vector.tensor_add(v1T_sb[:], v1T_ps[:], bvT[:])

        # broadcast each v1T column across the free dim for stage-2 lhsT
        v1bc_sb = small.tile([128, D], f32)
        nc.vector.tensor_copy(
            v1bc_sb[:].rearrange("p (k m) -> p k m", k=KC),
            v1T_sb[:, :, None].broadcast_to([128, KC, 128]),
        )

        # ---- stage 2 on PE: out quarter (broadcast) = v1 @ WoT[:, q] + bo ----
        acc = psum2.tile([128, QW], f32)
        for k in range(KC):
            nc.tensor.matmul(
                acc[:],
                v1bc_sb[:, k * 128 : (k + 1) * 128],
                woq_sb[:, k * QW : (k + 1) * QW],
                start=(k == 0),
                stop=False,
            )
        nc.tensor.matmul(acc[:], ones1x128[:], boq_sb[:], start=False, stop=True)
        out_sb = outp.tile([128, QW], f32)
        nc.vector.tensor_copy(out_sb[:], acc[:])

        # ---- broadcast-write the row quarter across all 2048 seq rows ----
        # split across both HWDGE rings for write bandwidth
        hs = S // 2
        nc.sync.dma_start(
            y[0:hs, :].rearrange("(a p) c -> p a c", p=128),
            out_sb[:, None, :].broadcast_to([128, hs // 128, QW]),
        )
        nc.scalar.dma_start(
            y[hs:S, :].rearrange("(a p) c -> p a c", p=128),
            out_sb[:, None, :].broadcast_to([128, hs // 128, QW]),
        )

    nc.compile()
    return nc


def _prep_inputs(condition, Wv, bv, Wo, bo):
    cond = np.asarray(condition, np.float32)
    wvt = np.ascontiguousarray(
        np.asarray(Wv, np.float32).T.reshape(KC, 128, D).transpose(1, 0, 2).reshape(128, KC * D)
    )
    WoT = np.asarray(Wo, np.float32).T  # [d, dout]
    bvT = np.asarray(bv, np.float32).reshape(KC, 128).T  # [128, KC]
    bo_ = np.asarray(bo, np.float32)
    smalls = []
    for b in range(B):
        condT = cond[b].reshape(KC, 128).T  # [128, KC]
        smalls.append(np.ascontiguousarray(np.concatenate([condT, bvT], axis=1)))
    woqs, boqs = [], []
    for q in range(QCORES):
        sl = WoT[:, q * QW : (q + 1) * QW]  # [1024, 256]
        woqs.append(
            np.ascontiguousarray(
                sl.reshape(KC, 128, QW).transpose(1, 0, 2).reshape(128, KC * QW)
            )
        )
        boqs.append(np.ascontiguousarray(bo_[q * QW : (q + 1) * QW].reshape(1, QW)))
    in_maps = []
    for i in range(NCORES):
        b, q = i // QCORES, i % QCORES
        in_maps.append(
            {"smalls": smalls[b], "wvp": wvt, "woq": woqs[q], "boq": boqs[q]}
        )
    return in_maps


def _run(in_maps, **kwargs):
    if "nc" not in _cache:
        _cache["nc"] = _build()
    return run_bass_kernel_spmd(
        _cache["nc"], in_maps, core_ids=list(range(NCORES)), **kwargs
    )


def kernel(hidden_states, condition, Wq, bq, Wk, bk, Wv, bv, Wo, bo):
    in_maps = _prep_inputs(condition, Wv, bv, Wo, bo)
    res = _run(in_maps)
    full = np.empty((B, S, D), np.float32)
    for i in range(NCORES):
        b, q = i // QCORES, i % QCORES
        full[b, :, q * QW : (q + 1) * QW] = np.asarray(res.results[i]["y"])
    return full
